# revision 20
# baseline (speedup 1.0000x reference)
"""Trainium2 Bass kernel for nn_LinearLLM: out[b,t,v] = sum_{s>=t,w} x[b,s,w]*W[s,w,t,v] + bias.

Algebraic reduction: x[b,s,:] = embedding[src[b,s]] takes only V=6 values, so
the EMB=64 contraction is folded into the weight ON HOST:
    W2[(s,k),(t,v)] = sum_w emb[k,w] * weight[s,w,t,v] * mask(s>=t)
and the device computes a single one-hot matmul
    out[b,(t,v)] = sum_{(s,k)} onehot[b,(s,k)] * W2[(s,k),(t,v)]
with contraction K = L1*V = 3078 (25 chunks of 128) instead of L1*EMB = 32832.

Sharding: t-axis cyclic over 8 cores (core c owns t in {c, c+8, ...}) so the
causal prefix-width per K-chunk is uniform across cores -> one SPMD program.
Per-core DMA: ~0.7MB W2 slab + ~0.4MB one-hot + 0.1MB out (fp8) vs ~21MB for
the dense (s,w) formulation.

dtype: float8 e3m4 (4 mantissa bits), W2 pre-scaled by 64 so values land in
the normal range; one-hot 1.0 is exact in fp8. Measured end-to-end rel err
~1.4e-2 (vs 2e-2 tolerance). Set FP8=False for a bf16 fallback (~2e-3).

K-chunks are issued in DESCENDING order (24 first, full 390-col width with
start=True, so no zero-init matmul is needed).
"""
import numpy as np
import ml_dtypes

from concourse import bacc, tile
from concourse.bass_utils import run_bass_kernel_spmd
import concourse.mybir as mybir

B, L1, EMB, V, NCORES = 128, 513, 64, 6, 8
CNT = 65                       # padded t-count per core (core 0 has 65)
NCOLS = CNT * V                # 390 output columns per core
NROWS = L1 * V                 # 3078 contraction rows (s,k)
NCHUNK = 25                    # ceil(3078/128) K-chunks of 128
NROWS_PAD = NCHUNK * 128       # 3200

FP8 = True
if FP8:
    MM_DT = mybir.dt.float8e3
    NP_DT = ml_dtypes.float8_e3m4
    SCALE = 64.0
else:
    MM_DT = mybir.dt.bfloat16
    NP_DT = ml_dtypes.bfloat16
    SCALE = 1.0


def _width(j):
    """Masked column-prefix width for K-chunk j (core-0 worst case)."""
    s_max = min(L1 - 1, (128 * (j + 1) - 1) // V)
    return 6 * min(CNT, s_max // 8 + 1)


# DMA groups of K-chunks. Chunk 24 holds only rows 3072..3077 (s=512, the
# rest is padding) so it is trimmed to K=6 partitions -- a 3KB DMA that
# lands first and opens the PSUM accumulation (start=True, full 390 width).
#
# Measured DMA behaviour on this part drives the layout: every dma_start has
# a multi-us serialized cost on its issue channel, and there are exactly
# three channels (SWDGE/gpsimd ~180GB/s once streaming, and the two HWDGE
# rings which SHARE one ~97GB/s generator). Groups are placed so each
# channel streams serially while PE consumes them in arrival order.
# (chunks, channel): channel 0 = gpsimd, 1 = sync/SP, 2 = scalar/ACT
# listed in PE-consumption order (expected arrival order); accumulation
# order into PSUM is irrelevant, only chunk 24 must come first (start=True)
GROUPS = [
    ([24], 1),                       # 3KB; opens accumulation; SP ring
    (list(range(18, 12, -1)), 2),    # 18..13 on ACT: its only DMA, lands 2nd
    (list(range(23, 18, -1)), 1),    # 23..19 on SP behind the tiny opener
    (list(range(12, 5, -1)), 0),     # 12..6
    (list(range(5, -1, -1)), 0),     # 5..0
]
assert sorted(j for g, _ in GROUPS for j in g) == list(range(NCHUNK))


def _kdim(j):
    return 6 if j == NCHUNK - 1 else 128


def _group_width(chunks):
    return sum(128 + _width(j) for j in chunks)

_CACHE = {}


def _build():
    if "nc" in _CACHE:
        return _CACHE["nc"]
    nc = bacc.Bacc("TRN2", target_bir_lowering=False, debug=False,
                   num_devices=NCORES)
    g_dram = [nc.declare_dram_parameter(f"g{i}", [_kdim(g[0]),
                                                  _group_width(g)],
                                        MM_DT, isOutput=False)
              for i, (g, _) in enumerate(GROUPS)]
    out_dram = nc.declare_dram_parameter("out", [128, NCOLS],
                                         mybir.dt.float16, isOutput=True)

    with tile.TileContext(nc) as tc:
        with (
            tc.tile_pool(name="op", bufs=1) as op,
            tc.tile_pool(name="psum", bufs=1, space="PSUM") as psp,
        ):
            ps = psp.tile([128, NCOLS], mybir.dt.float32)
            chans = [nc.gpsimd, nc.sync, nc.scalar]
            tiles = []
            # issue every group DMA up front on its channel (distinct tags:
            # same-named tiles share one pool slot and would serialize)
            for i, (g, ch) in enumerate(GROUPS):
                t = op.tile([_kdim(g[0]), _group_width(g)], MM_DT,
                            tag=f"grp{i}", name=f"grp{i}")
                chans[ch].dma_start(t[:], g_dram[i][:])
                tiles.append(t)

            # Warm-up: ~3.4us of dummy matmuls while the first DMAs are in
            # flight trips the PE HAM clock gate (1.2 -> 2.4 GHz) before the
            # real accumulation starts.
            warm = op.tile([128, 512], MM_DT, name="warm")
            nc.gpsimd.memset(warm[:], 0.0)
            pwarm = psp.tile([128, 512], mybir.dt.float32, name="pwarm")
            for _ in range(8):
                nc.tensor.matmul(pwarm[:], warm[:, :128], warm[:],
                                 start=True, stop=True)

            for i, (g, _) in enumerate(GROUPS):
                t = tiles[i]
                base = 128 * len(g)
                ok = 0
                for idx, j in enumerate(g):
                    wj = _width(j)
                    nc.tensor.matmul(ps[:, :wj],
                                     t[:, idx * 128:(idx + 1) * 128],
                                     t[:, base + ok:base + ok + wj],
                                     start=(j == NCHUNK - 1),
                                     stop=(j == 0))
                    ok += wj

            o = op.tile([128, NCOLS], mybir.dt.float16)
            nc.vector.tensor_copy(o[:], ps[:])
            nc.gpsimd.dma_start(out_dram[:], o[:])

    nc.compile()
    _CACHE["nc"] = nc
    return nc


def _prep_inputs(src, embedding, weight):
    src = np.asarray(src)
    emb = np.asarray(embedding, dtype=np.float32)
    weight = np.asarray(weight, dtype=np.float32)

    # one-hot lhsT, layout oh[p, j*128 + b] = 1 iff src[b, r//6] == r%6
    # with r = 128j + p  (shared by all cores)
    oh = np.zeros((128, NROWS_PAD), np.float32)
    r = np.arange(L1)[None, :] * V + src            # (B, L1)
    p = r % 128
    cols = (r // 128) * 128 + np.arange(B)[:, None]
    oh[p.ravel(), cols.ravel()] = 1.0
    oh = oh.astype(NP_DT)

    # W2[(s,k), (t,v)] = sum_w emb[k,w] * weight[s,w,t,v]
    W2 = np.matmul(emb[None], weight.reshape(L1, EMB, L1 * V))  # (513, 6, 3078)
    W2 = W2.reshape(NROWS, L1 * V)
    svals = np.arange(NROWS) // V

    in_maps = []
    for c in range(NCORES):
        tvals = np.arange(c, L1, 8)
        cnt = len(tvals)
        cols_c = (tvals[:, None] * V + np.arange(V)[None, :]).ravel()
        Wc = W2[:, cols_c] * (svals[:, None] >= np.repeat(tvals, V)[None, :])
        Wp = np.zeros((NROWS_PAD, NCOLS), np.float32)
        Wp[:NROWS, :cnt * V] = Wc
        q = (Wp * SCALE).astype(NP_DT)
        in_map = {}
        for i, (g, _) in enumerate(GROUPS):
            kd = _kdim(g[0])
            blocks = [oh[:kd, 128 * j:128 * (j + 1)] for j in g]
            blocks += [q[128 * j:128 * j + kd, :_width(j)] for j in g]
            in_map[f"g{i}"] = np.ascontiguousarray(
                np.concatenate(blocks, axis=1))
        in_maps.append(in_map)
    return in_maps


def _unshard(results, bias):
    full = np.zeros((B, L1, V), np.float32)
    for c in range(NCORES):
        cnt = len(range(c, L1, 8))
        oc = results[c]["out"].astype(np.float32).reshape(B, CNT, V)
        full[:, c::8, :] = oc[:, :cnt, :] / SCALE
    full += np.asarray(bias, dtype=np.float32)[None]
    return np.ascontiguousarray(full.transpose(0, 2, 1))


def kernel(src, embedding, weight, bias):
    nc = _build()
    in_maps = _prep_inputs(src, embedding, weight)
    res = run_bass_kernel_spmd(nc, in_maps, list(range(NCORES)))
    return _unshard(res.results, bias)


# revision 22
# speedup vs baseline: 1.0974x; 1.0974x over previous
"""Trainium2 Bass kernel for nn_LinearLLM: out[b,t,v] = sum_{s>=t,w} x[b,s,w]*W[s,w,t,v] + bias.

Algebraic reduction: x[b,s,:] = embedding[src[b,s]] takes only V=6 values, so
the EMB=64 contraction is folded into the weight ON HOST:
    W2[(s,k),(t,v)] = sum_w emb[k,w] * weight[s,w,t,v] * mask(s>=t)
and the device computes a single one-hot matmul
    out[b,(t,v)] = sum_{(s,k)} onehot[b,(s,k)] * W2[(s,k),(t,v)]
with contraction K = L1*V = 3078 (25 chunks of 128) instead of L1*EMB = 32832.

Sharding: t-axis cyclic over 8 cores (core c owns t in {c, c+8, ...}) so the
causal prefix-width per K-chunk is uniform across cores -> one SPMD program.
Per-core DMA: ~0.7MB W2 slab + ~0.4MB one-hot + 0.1MB out (fp8) vs ~21MB for
the dense (s,w) formulation.

dtype: float8 e3m4 (4 mantissa bits), W2 pre-scaled by 64 so values land in
the normal range; one-hot 1.0 is exact in fp8. Measured end-to-end rel err
~1.4e-2 (vs 2e-2 tolerance). Set FP8=False for a bf16 fallback (~2e-3).

K-chunks are issued in DESCENDING order (24 first, full 390-col width with
start=True, so no zero-init matmul is needed).
"""
import numpy as np
import ml_dtypes

from concourse import bacc, tile
from concourse.bass_utils import run_bass_kernel_spmd
import concourse.mybir as mybir

B, L1, EMB, V, NCORES = 128, 513, 64, 6, 8
CNT = 65                       # padded t-count per core (core 0 has 65)
NCOLS = CNT * V                # 390 output columns per core
NROWS = L1 * V                 # 3078 contraction rows (s,k)
NCHUNK = 25                    # ceil(3078/128) K-chunks of 128
NROWS_PAD = NCHUNK * 128       # 3200

FP8 = True
if FP8:
    MM_DT = mybir.dt.float8e3
    NP_DT = ml_dtypes.float8_e3m4
    SCALE = 64.0
else:
    MM_DT = mybir.dt.bfloat16
    NP_DT = ml_dtypes.bfloat16
    SCALE = 1.0


def _width(j):
    """Masked column-prefix width for K-chunk j (core-0 worst case)."""
    s_max = min(L1 - 1, (128 * (j + 1) - 1) // V)
    return 6 * min(CNT, s_max // 8 + 1)


# DMA groups of K-chunks. Chunk 24 holds only rows 3072..3077 (s=512, the
# rest is padding) so it is trimmed to K=6 partitions -- a 3KB DMA that
# lands first and opens the PSUM accumulation (start=True, full 390 width).
#
# Measured DMA behaviour on this part drives the layout: every dma_start has
# a multi-us serialized cost on its issue channel, and there are exactly
# three channels (SWDGE/gpsimd ~180GB/s once streaming, and the two HWDGE
# rings which SHARE one ~97GB/s generator). Groups are placed so each
# channel streams serially while PE consumes them in arrival order.
# (chunks, channel): channel 0 = gpsimd, 1 = sync/SP, 2 = scalar/ACT
# listed in PE-consumption order (expected arrival order); accumulation
# order into PSUM is irrelevant, only chunk 24 must come first (start=True)
GROUPS = [
    ([24], 1),                       # 3KB; opens accumulation; SP ring
    (list(range(18, 12, -1)), 2),    # 18..13 on ACT: first ACT DMA, lands 2nd
    (list(range(23, 18, -1)), 1),    # 23..19 on SP behind the tiny opener
    (list(range(12, 5, -1)), 2),     # 12..6 on ACT behind 18..13
    (list(range(5, -1, -1)), 0),     # 5..0 on SWDGE
]
assert sorted(j for g, _ in GROUPS for j in g) == list(range(NCHUNK))


def _kdim(j):
    return 6 if j == NCHUNK - 1 else 128


def _group_width(chunks):
    return sum(128 + _width(j) for j in chunks)

_CACHE = {}


def _build():
    if "nc" in _CACHE:
        return _CACHE["nc"]
    nc = bacc.Bacc("TRN2", target_bir_lowering=False, debug=False,
                   num_devices=NCORES)
    g_dram = [nc.declare_dram_parameter(f"g{i}", [_kdim(g[0]),
                                                  _group_width(g)],
                                        MM_DT, isOutput=False)
              for i, (g, _) in enumerate(GROUPS)]
    out_dram = nc.declare_dram_parameter("out", [128, NCOLS],
                                         mybir.dt.float16, isOutput=True)

    with tile.TileContext(nc) as tc:
        with (
            tc.tile_pool(name="op", bufs=1) as op,
            tc.tile_pool(name="psum", bufs=1, space="PSUM") as psp,
        ):
            ps = psp.tile([128, NCOLS], mybir.dt.float32)
            chans = [nc.gpsimd, nc.sync, nc.scalar]

            # Warm-up: ~3.4us of dummy matmuls while the first DMAs are in
            # flight trips the PE HAM clock gate (1.2 -> 2.4 GHz) before the
            # real accumulation starts. The memset is emitted BEFORE the
            # SWDGE descriptor-gen instructions so gpsimd runs it first.
            warm = op.tile([128, 512], MM_DT, name="warm")
            nc.gpsimd.memset(warm[:], 0.0)

            tiles = []
            # issue every group DMA up front on its channel (distinct tags:
            # same-named tiles share one pool slot and would serialize)
            for i, (g, ch) in enumerate(GROUPS):
                t = op.tile([_kdim(g[0]), _group_width(g)], MM_DT,
                            tag=f"grp{i}", name=f"grp{i}")
                chans[ch].dma_start(t[:], g_dram[i][:])
                tiles.append(t)

            pwarm = psp.tile([128, 512], mybir.dt.float32, name="pwarm")
            for _ in range(8):
                nc.tensor.matmul(pwarm[:], warm[:, :128], warm[:],
                                 start=True, stop=True)

            for i, (g, _) in enumerate(GROUPS):
                t = tiles[i]
                base = 128 * len(g)
                ok = 0
                for idx, j in enumerate(g):
                    wj = _width(j)
                    nc.tensor.matmul(ps[:, :wj],
                                     t[:, idx * 128:(idx + 1) * 128],
                                     t[:, base + ok:base + ok + wj],
                                     start=(j == NCHUNK - 1),
                                     stop=(j == 0))
                    ok += wj

            o = op.tile([128, NCOLS], mybir.dt.float16)
            nc.vector.tensor_copy(o[:], ps[:])
            nc.gpsimd.dma_start(out_dram[:], o[:])

    nc.compile()
    _CACHE["nc"] = nc
    return nc


def _prep_inputs(src, embedding, weight):
    src = np.asarray(src)
    emb = np.asarray(embedding, dtype=np.float32)
    weight = np.asarray(weight, dtype=np.float32)

    # one-hot lhsT, layout oh[p, j*128 + b] = 1 iff src[b, r//6] == r%6
    # with r = 128j + p  (shared by all cores)
    oh = np.zeros((128, NROWS_PAD), np.float32)
    r = np.arange(L1)[None, :] * V + src            # (B, L1)
    p = r % 128
    cols = (r // 128) * 128 + np.arange(B)[:, None]
    oh[p.ravel(), cols.ravel()] = 1.0
    oh = oh.astype(NP_DT)

    # W2[(s,k), (t,v)] = sum_w emb[k,w] * weight[s,w,t,v]
    W2 = np.matmul(emb[None], weight.reshape(L1, EMB, L1 * V))  # (513, 6, 3078)
    W2 = W2.reshape(NROWS, L1 * V)
    svals = np.arange(NROWS) // V

    in_maps = []
    for c in range(NCORES):
        tvals = np.arange(c, L1, 8)
        cnt = len(tvals)
        cols_c = (tvals[:, None] * V + np.arange(V)[None, :]).ravel()
        Wc = W2[:, cols_c] * (svals[:, None] >= np.repeat(tvals, V)[None, :])
        Wp = np.zeros((NROWS_PAD, NCOLS), np.float32)
        Wp[:NROWS, :cnt * V] = Wc
        q = (Wp * SCALE).astype(NP_DT)
        in_map = {}
        for i, (g, _) in enumerate(GROUPS):
            kd = _kdim(g[0])
            blocks = [oh[:kd, 128 * j:128 * (j + 1)] for j in g]
            blocks += [q[128 * j:128 * j + kd, :_width(j)] for j in g]
            in_map[f"g{i}"] = np.ascontiguousarray(
                np.concatenate(blocks, axis=1))
        in_maps.append(in_map)
    return in_maps


def _unshard(results, bias):
    full = np.zeros((B, L1, V), np.float32)
    for c in range(NCORES):
        cnt = len(range(c, L1, 8))
        oc = results[c]["out"].astype(np.float32).reshape(B, CNT, V)
        full[:, c::8, :] = oc[:, :cnt, :] / SCALE
    full += np.asarray(bias, dtype=np.float32)[None]
    return np.ascontiguousarray(full.transpose(0, 2, 1))


def kernel(src, embedding, weight, bias):
    nc = _build()
    in_maps = _prep_inputs(src, embedding, weight)
    res = run_bass_kernel_spmd(nc, in_maps, list(range(NCORES)))
    return _unshard(res.results, bias)
